# revision 3
# baseline (speedup 1.0000x reference)
"""GraphSAGE 2-layer encoder + score kernel for 8 Trainium2 NeuronCores.

Strategy (two SPMD launches):
  Phase 1 (data-parallel over node tables): each core computes
  h0 = relu([self, mean(neigh)] @ W0.T) for its 1/8 slice of the 20000-row
  center table and 60000-row contexts/negatives table. Self features are
  host-pre-transposed to feature-major so they can feed the PE as lhsT with
  no on-device transpose. Neighbor rows are contiguous in DRAM -> 1MB DMAs;
  the mean over 16 neighbors is a strided free-axis DVE reduce; agg is
  PE-transposed to feature-major; two accumulating matmuls + relu.

  Host: concatenate the 8 h0 shards into full tables (not device work).

  Phase 2 (data-parallel over the 4096-row frontier): each core takes 512
  center rows and the matching 3072 context rows. For each 128-row tile one
  indirect DMA gathers 17 rows per partition (self + 16 neighbors) from the
  full h0 table; mean/transpose/matmul/relu as above. Scores
  center[b] . cn[b,c] are computed with a fixed selection-matrix matmul that
  replicates each center row 6x across partitions, then a DVE mul + reduce.
"""

import numpy as np

import concourse.bass as bass
import concourse.mybir as mybir
from concourse import bacc
from concourse.bass import IndirectOffsetOnAxis, ts
from concourse.bass_utils import run_bass_kernel_spmd
from concourse.masks import make_identity
from concourse.tile import TileContext

F32 = mybir.dt.float32
I32 = mybir.dt.int32

NCORES = 8
D = 128          # feature dim (also layer-0/1 output dim)
KN = 16          # neighbors per node
N0C = 20000      # center table rows
N0N = 60000      # contexts/negatives table rows
B = 4096         # center frontier rows
C = 6            # contexts+negatives per center row

CSH = N0C // NCORES      # 2500 center table rows per core
NSH = N0N // NCORES      # 7500 cn table rows per core
CSH_P = 2560             # padded to multiple of 128 (20 tiles)
NSH_P = 7552             # padded to multiple of 128 (59 tiles)
BC = B // NCORES         # 512 center frontier rows per core (4 tiles)
BN = B * C // NCORES     # 3072 cn frontier rows per core (24 tiles)

_CACHE = {}
last_exec_times = []     # exec_time_ns per launch of the most recent kernel() call


def _build_phase1():
    nc = bacc.Bacc("TRN2", num_devices=NCORES)
    selfT_c = nc.declare_dram_parameter("selfT_c", [D, CSH_P], F32, isOutput=False)
    neigh_c = nc.declare_dram_parameter("neigh_c", [CSH_P * KN, D], F32, isOutput=False)
    selfT_n = nc.declare_dram_parameter("selfT_n", [D, NSH_P], F32, isOutput=False)
    neigh_n = nc.declare_dram_parameter("neigh_n", [NSH_P * KN, D], F32, isOutput=False)
    w0aT = nc.declare_dram_parameter("w0aT", [D, D], F32, isOutput=False)
    w0bT = nc.declare_dram_parameter("w0bT", [D, D], F32, isOutput=False)
    h0_c = nc.declare_dram_parameter("h0_c", [CSH_P, D], F32, isOutput=True)
    h0_n = nc.declare_dram_parameter("h0_n", [NSH_P, D], F32, isOutput=True)

    with TileContext(nc) as tc:
        with (
            tc.tile_pool(name="const", bufs=1) as constp,
            tc.tile_pool(name="io", bufs=3) as iop,
            tc.tile_pool(name="mid", bufs=3) as midp,
            tc.tile_pool(name="ps", bufs=2, space="PSUM") as psp,
        ):
            ident = constp.tile([D, D], F32, tag="ident")
            make_identity(nc, ident[:])
            wa = constp.tile([D, D], F32, tag="wa")
            nc.sync.dma_start(out=wa[:], in_=w0aT[:])
            wb = constp.tile([D, D], F32, tag="wb")
            nc.sync.dma_start(out=wb[:], in_=w0bT[:])

            for selfT, neigh, hout, ntiles in (
                (selfT_c, neigh_c, h0_c, CSH_P // 128),
                (selfT_n, neigh_n, h0_n, NSH_P // 128),
            ):
                neigh_tiles = neigh[:].rearrange("(t p j) d -> t p (j d)", p=128, j=KN)
                for t in range(ntiles):
                    big = iop.tile([128, KN * D], F32, tag="big")
                    nc.sync.dma_start(out=big[:], in_=neigh_tiles[t])
                    agg = midp.tile([128, D], F32, tag="agg")
                    nc.vector.tensor_reduce(
                        out=agg[:],
                        in_=big[:].rearrange("p (j d) -> p d j", j=KN),
                        axis=mybir.AxisListType.X,
                        op=mybir.AluOpType.add,
                    )
                    aggT_ps = psp.tile([128, D], F32, tag="tr")
                    nc.tensor.transpose(aggT_ps[:], agg[:], ident[:])
                    aggT = midp.tile([128, D], F32, tag="aggT")
                    nc.vector.tensor_copy(out=aggT[:], in_=aggT_ps[:])
                    sT = iop.tile([128, D], F32, tag="sT")
                    nc.sync.dma_start(out=sT[:], in_=selfT[:, ts(t, 128)])
                    h_ps = psp.tile([128, D], F32, tag="h")
                    nc.tensor.matmul(h_ps[:], lhsT=sT[:], rhs=wa[:], start=True, stop=False)
                    nc.tensor.matmul(h_ps[:], lhsT=aggT[:], rhs=wb[:], start=False, stop=True)
                    h_sb = midp.tile([128, D], F32, tag="h_sb")
                    nc.scalar.activation(h_sb[:], h_ps[:], mybir.ActivationFunctionType.Relu)
                    nc.sync.dma_start(out=hout[ts(t, 128), :], in_=h_sb[:])
    nc.compile()
    return nc


def _build_phase2():
    nc = bacc.Bacc("TRN2", num_devices=NCORES)
    tab_c = nc.declare_dram_parameter("tab_c", [N0C, D], F32, isOutput=False)
    tab_n = nc.declare_dram_parameter("tab_n", [N0N, D], F32, isOutput=False)
    idx_c = nc.declare_dram_parameter("idx_c", [128, (BC // 128) * (KN + 1)], I32, isOutput=False)
    idx_n = nc.declare_dram_parameter("idx_n", [128, (BN // 128) * (KN + 1)], I32, isOutput=False)
    w1aT = nc.declare_dram_parameter("w1aT", [D, D], F32, isOutput=False)
    w1bT = nc.declare_dram_parameter("w1bT", [D, D], F32, isOutput=False)
    smask = nc.declare_dram_parameter("smask", [C, 128, 128], F32, isOutput=False)
    h1_c = nc.declare_dram_parameter("h1_c", [BC, D], F32, isOutput=True)
    scores = nc.declare_dram_parameter("scores", [128, BN // 128], F32, isOutput=True)

    n_ct = BC // 128   # 4 center tiles
    n_nt = BN // 128   # 24 cn tiles (6 per center tile)
    G = KN + 1         # rows gathered per partition: self + 16 neighbors

    with TileContext(nc) as tc:
        with (
            tc.tile_pool(name="const", bufs=1) as constp,
            tc.tile_pool(name="io", bufs=3) as iop,
            tc.tile_pool(name="mid", bufs=3) as midp,
            tc.tile_pool(name="ctr", bufs=2) as ctrp,
            tc.tile_pool(name="ps", bufs=2, space="PSUM") as psp,
        ):
            ident = constp.tile([D, D], F32, tag="ident")
            make_identity(nc, ident[:])
            wa = constp.tile([D, D], F32, tag="wa")
            nc.sync.dma_start(out=wa[:], in_=w1aT[:])
            wb = constp.tile([D, D], F32, tag="wb")
            nc.sync.dma_start(out=wb[:], in_=w1bT[:])
            sm = []
            for k in range(C):
                t_ = constp.tile([128, 128], F32, tag=f"sm{k}")
                nc.sync.dma_start(out=t_[:], in_=smask[k])
                sm.append(t_)
            idxc_sb = constp.tile([128, n_ct * G], I32, tag="idxc")
            nc.sync.dma_start(out=idxc_sb[:], in_=idx_c[:])
            idxn_sb = constp.tile([128, n_nt * G], I32, tag="idxn")
            nc.sync.dma_start(out=idxn_sb[:], in_=idx_n[:])
            sc_sb = constp.tile([128, n_nt], F32, tag="scout")

            def frontier_tile(tab, idx_sb, t):
                """Gather + SAGE layer for one 128-row frontier tile.
                Returns post-relu [128 rows, 128 feat] SBUF tile (from `pool`)."""
                big = iop.tile([128, G * D], F32, tag="big")
                for g in range(G):
                    nc.gpsimd.indirect_dma_start(
                        out=big[:, g * D : (g + 1) * D],
                        out_offset=None,
                        in_=tab[:],
                        in_offset=IndirectOffsetOnAxis(
                            ap=idx_sb[:, t * G + g : t * G + g + 1], axis=0
                        ),
                    )
                agg = midp.tile([128, D], F32, tag="agg")
                nc.vector.tensor_reduce(
                    out=agg[:],
                    in_=big[:, D:].rearrange("p (j d) -> p d j", j=KN),
                    axis=mybir.AxisListType.X,
                    op=mybir.AluOpType.add,
                )
                aggT_ps = psp.tile([128, D], F32, tag="tr")
                nc.tensor.transpose(aggT_ps[:], agg[:], ident[:])
                aggT = midp.tile([128, D], F32, tag="aggT")
                nc.vector.tensor_copy(out=aggT[:], in_=aggT_ps[:])
                sT_ps = psp.tile([128, D], F32, tag="tr")
                nc.tensor.transpose(sT_ps[:], big[:, :D], ident[:])
                sT = midp.tile([128, D], F32, tag="sT")
                nc.vector.tensor_copy(out=sT[:], in_=sT_ps[:])
                h_ps = psp.tile([128, D], F32, tag="h")
                nc.tensor.matmul(h_ps[:], lhsT=sT[:], rhs=wa[:], start=True, stop=False)
                nc.tensor.matmul(h_ps[:], lhsT=aggT[:], rhs=wb[:], start=False, stop=True)
                return h_ps

            for ct in range(n_ct):
                h_ps = frontier_tile(tab_c, idxc_sb, ct)
                ctr = ctrp.tile([128, D], F32, tag="ctr")
                nc.scalar.activation(ctr[:], h_ps[:], mybir.ActivationFunctionType.Relu)
                nc.sync.dma_start(out=h1_c[ts(ct, 128), :], in_=ctr[:])
                for k in range(C):
                    t = ct * C + k
                    h_ps2 = frontier_tile(tab_n, idxn_sb, t)
                    cn_sb = midp.tile([128, D], F32, tag="cn")
                    nc.scalar.activation(cn_sb[:], h_ps2[:], mybir.ActivationFunctionType.Relu)
                    rep_ps = psp.tile([128, D], F32, tag="rep")
                    nc.tensor.matmul(rep_ps[:], lhsT=sm[k][:], rhs=ctr[:], start=True, stop=True)
                    prod = midp.tile([128, D], F32, tag="prod")
                    nc.vector.tensor_tensor(
                        out=prod[:], in0=cn_sb[:], in1=rep_ps[:], op=mybir.AluOpType.mult
                    )
                    nc.vector.tensor_reduce(
                        out=sc_sb[:, t : t + 1],
                        in_=prod[:],
                        axis=mybir.AxisListType.X,
                        op=mybir.AluOpType.add,
                    )
            nc.sync.dma_start(out=scores[:], in_=sc_sb[:])
    nc.compile()
    return nc


def _get(name, builder):
    if name not in _CACHE:
        _CACHE[name] = builder()
    return _CACHE[name]


def kernel(**inputs):
    global last_exec_times
    last_exec_times = []

    cf = np.ascontiguousarray(np.asarray(inputs["center_feats_data"], dtype=np.float32))
    cnf = np.asarray(inputs["center_neigh_feats_data"], dtype=np.float32).reshape(N0C * KN, D)
    nf = np.ascontiguousarray(np.asarray(inputs["contexts_negatives_feats_data"], dtype=np.float32))
    nnf = np.asarray(inputs["contexts_negatives_neigh_feats_data"], dtype=np.float32).reshape(N0N * KN, D)
    cmap = np.asarray(inputs["center_nodes_map"]).astype(np.int32).reshape(B)
    cnmap = np.asarray(inputs["center_neigh_nodes_map"]).astype(np.int32).reshape(B, KN)
    nmap = np.asarray(inputs["contexts_negatives_nodes_map"]).astype(np.int32).reshape(B * C)
    nnmap = np.asarray(inputs["contexts_negatives_neigh_nodes_map"]).astype(np.int32).reshape(B * C, KN)
    W0 = np.asarray(inputs["W0"], dtype=np.float32)
    W1 = np.asarray(inputs["W1"], dtype=np.float32)

    w0aT = np.ascontiguousarray(W0[:, :D].T)
    w0bT = np.ascontiguousarray(W0[:, D:].T) / KN   # fold the neighbor mean into the weight
    w1aT = np.ascontiguousarray(W1[:, :D].T)
    w1bT = np.ascontiguousarray(W1[:, D:].T) / KN

    cfT = cf.T  # [D, N0C]
    nfT = nf.T

    in1 = []
    for i in range(NCORES):
        sc = np.zeros((D, CSH_P), np.float32)
        sc[:, :CSH] = cfT[:, i * CSH : (i + 1) * CSH]
        ngc = np.zeros((CSH_P * KN, D), np.float32)
        ngc[: CSH * KN] = cnf[i * CSH * KN : (i + 1) * CSH * KN]
        sn = np.zeros((D, NSH_P), np.float32)
        sn[:, :NSH] = nfT[:, i * NSH : (i + 1) * NSH]
        ngn = np.zeros((NSH_P * KN, D), np.float32)
        ngn[: NSH * KN] = nnf[i * NSH * KN : (i + 1) * NSH * KN]
        in1.append(
            {
                "selfT_c": sc,
                "neigh_c": ngc,
                "selfT_n": sn,
                "neigh_n": ngn,
                "w0aT": w0aT,
                "w0bT": w0bT,
            }
        )

    nc1 = _get("p1", _build_phase1)
    import os

    trace = bool(os.environ.get("BASS_TRACE"))
    r1 = run_bass_kernel_spmd(nc1, in1, list(range(NCORES)), trace=trace)
    last_exec_times.append(r1.exec_time_ns)
    h0c = np.concatenate([r1.results[i]["h0_c"][:CSH] for i in range(NCORES)], axis=0)
    h0n = np.concatenate([r1.results[i]["h0_n"][:NSH] for i in range(NCORES)], axis=0)

    # selection masks: rep[row] = center[(128*k + row)//6] for the 6 cn tiles
    # within one center tile's group of 768 rows
    smask = np.zeros((C, 128, 128), np.float32)
    for k in range(C):
        rows = np.arange(128)
        smask[k, (128 * k + rows) // 6, rows] = 1.0

    G = KN + 1
    in2 = []
    for i in range(NCORES):
        ic = np.zeros((128, (BC // 128) * G), np.int32)
        for t in range(BC // 128):
            rows = i * BC + t * 128 + np.arange(128)
            ic[:, t * G] = cmap[rows]
            ic[:, t * G + 1 : (t + 1) * G] = cnmap[rows]
        inn = np.zeros((128, (BN // 128) * G), np.int32)
        for t in range(BN // 128):
            rows = i * BN + t * 128 + np.arange(128)
            inn[:, t * G] = nmap[rows]
            inn[:, t * G + 1 : (t + 1) * G] = nnmap[rows]
        in2.append(
            {
                "tab_c": h0c,
                "tab_n": h0n,
                "idx_c": ic,
                "idx_n": inn,
                "w1aT": w1aT,
                "w1bT": w1bT,
                "smask": smask,
            }
        )

    nc2 = _get("p2", _build_phase2)
    r2 = run_bass_kernel_spmd(nc2, in2, list(range(NCORES)), trace=trace)
    last_exec_times.append(r2.exec_time_ns)

    center = np.concatenate([r2.results[i]["h1_c"] for i in range(NCORES)], axis=0)
    sc = np.concatenate(
        [r2.results[i]["scores"].T.reshape(BN) for i in range(NCORES)], axis=0
    )
    scores = sc.reshape(B, C)[:, None, :]
    return center, scores
